# revision 2
# baseline (speedup 1.0000x reference)
"""Trainium2 Bass kernel for nn_MultiHeadAttention_62371515073076.

Math (per batch b, faithful to the reference's softmax over the QUERY axis):
  q/k/v = einsum('nc,chd->nhd', x, W{q,k,v})
  s[i,j,h] = q[i,h,:].k[j,h,:] / 8
  attnw[i,h] = sum_j exp(s[i,j,h]) / Z[j,h],  Z[j,h] = sum_i exp(s[i,j,h])
  out = einsum('ihd,ohd->io', v * attnw, Wout)

Sharding: batch 8 -> one batch per NeuronCore (data parallel), weights
replicated.

Engine strategy (HW-microbenched costs per [128,1024] tile):
  - ACT direct-PSUM exp with fused Z row-sum (accum_out): 1631 ns — one
    instruction drains PSUM, exponentiates, and accumulates Z. 48 tiles.
  - DVE "Schraudolph" exp for the other 16 tiles: Q,K are pre-scaled by
    sqrt(A0/8) on the host so PSUM scores hold A0*s; one tensor_scalar_add
    (+B, int32 output) IS exp(s) in float bit-pattern (max rel err 3.9%,
    only 16/64 of the j-contributions -> ~0.5% on attnw). Z for those via
    DVE reduce_sum of the bitcast tile. Zero ACT cost.
  This balances ACT ~78us vs DVE ~76us (vs 105us for the all-ACT or
  bounce-based schedules).
  - attnw accumulated over j by PE: lhsT = (1/Z) broadcast via a stride-0
    AP, two heads col-packed, PSUM-accumulated across the 8 j-tiles.
    Schraudolph halves run this matmul in f32r (dtype must match rhs).
  - Output projection in two 4-chunk waves through the attnw PSUM slot.
"""
import numpy as np
from contextlib import ExitStack

import concourse.bass as bass
import concourse.mybir as mybir
import concourse.tile as tile
from concourse import bacc
from concourse.vector_clock import ScopedClock
from concourse.bass_utils import run_bass_kernel_spmd
import bass_rust

N_CORES = 8
B, N, C, H, D, O = 8, 1024, 256, 8, 64, 256
HD = H * D  # 512
FP32 = mybir.dt.float32
F32R = mybir.dt.float32r
I32 = mybir.dt.int32
F16 = mybir.dt.float16
BF16 = mybir.dt.bfloat16
EXP = mybir.ActivationFunctionType.Exp

# Schraudolph exp-as-int-add constants. PSUM holds A0*s (via host-side
# sqrt(A0/8) scaling of Wq and Wk); bitcast(int32(A0*s + B)) ~ exp(s).
A0 = float(2 ** 23) / float(np.log(2.0))  # 12102203.16
B_SCH = 1064877216.0                      # tuned: max rel err 3.86%
C_QK = float(np.sqrt(A0 / 8.0))

# (t, jt, half) tiles whose exp runs on DVE via Schraudolph (half 1 only).
SCH_JTS = (1, 3, 5, 7)

# colsum pipeline lag behind scores, in jt steps
LAG = 2

_MAXW = 1  # max sync waits this toolchain's walrus accepts per instruction


class _TC(tile.TileContext):
    """TileContext that splits semaphore waits one-per-instruction.

    The walrus build in this toolchain rejects any instruction carrying more
    than one sync wait; engines execute in order, so excess waits move onto
    same-engine NOPs emitted immediately before the instruction.
    """

    def _commit_instruction(self, inst, lazy_reg_writes: bool = True):
        si = inst.sync_info
        if (
            si is not None
            and si.on_wait
            and len(si.on_wait) > _MAXW
            and inst.engine != mybir.EngineType.Unassigned
        ):
            waits = list(si.on_wait)
            inst.sync_info = bass_rust.SyncInfo(
                on_wait=waits[-_MAXW:], on_update=list(si.on_update or [])
            )
            for i in range(0, len(waits) - _MAXW, _MAXW):
                nop = self.nc.engines[inst.engine].nop(nofuse=True, hint="waitsplit")
                nop.ins.sync_info = bass_rust.SyncInfo(
                    on_wait=waits[i : i + _MAXW], on_update=[]
                )
        return super()._commit_instruction(inst, lazy_reg_writes)

    def _drain_and_barrier(self, tick_clock, wait_clock):
        probe = self.nc.sync.drain()
        wait_clock.add_sem_waits(
            probe.ins, ScopedClock({None: tick_clock.global_clock})
        )
        si = probe.ins.sync_info
        waits = list(si.on_wait or []) if si is not None else []
        if len(waits) > 1:
            probe.ins.sync_info = bass_rust.SyncInfo(
                on_wait=waits[:1], on_update=list(si.on_update or [])
            )
            for i in range(1, len(waits)):
                d = self.nc.sync.drain()
                d.ins.sync_info = bass_rust.SyncInfo(
                    on_wait=waits[i : i + 1], on_update=[]
                )
        self.nc.all_engine_barrier()
        assert self.sems is not None
        popped = self.nc._tile_sem_poison_stack.pop()
        assert popped is self._sem_poison
        self.nc.clear_and_free_semaphores(list(self.sems.allocated().values()))
        self.nc.all_engine_barrier()


def _bcast64(col_ap):
    """[P,1] AP -> [P,64] AP reading the same element 64x (free step 0)."""
    return bass.AP(col_ap.tensor, col_ap.offset, [list(col_ap.ap[0]), [0, 64]])


def _emit_body(tc, pools, xt, wqkv, wot, out):
    nc = tc.nc
    (wpool, qkvpool, gpool, g32pool, rzpool, rz32pool, zpool, obpool,
     sps, awps) = pools

    XT = [[None, None], [None, None]]
    WQC = [[None, None], [None, None], [None, None]]
    WOT = []

    def load_x(kc, ic):
        t = wpool.tile([128, 512], F16, tag=f"xt{kc}{ic}", name=f"xt{kc}{ic}")
        nc.sync.dma_start(
            t[:], xt[kc * 128 : (kc + 1) * 128, ic * 512 : (ic + 1) * 512]
        )
        XT[kc][ic] = t

    def load_w(col, kc):
        w = wpool.tile([128, HD], F16, tag=f"w{col}{kc}", name=f"w{col}{kc}")
        nc.sync.dma_start(
            w[:], wqkv[kc * 128 : (kc + 1) * 128, col * HD : (col + 1) * HD]
        )
        WQC[col][kc] = w

    load_x(0, 0); load_x(1, 0); load_w(0, 0); load_w(0, 1)
    load_x(0, 1); load_x(1, 1); load_w(1, 0); load_w(1, 1)
    load_w(2, 0); load_w(2, 1)
    for kt in range(4):
        w = wpool.tile([128, O], F16, tag=f"wot{kt}", name=f"wot{kt}")
        nc.sync.dma_start(w[:], wot[kt * 128 : (kt + 1) * 128, :])
        WOT.append(w)

    QT = [qkvpool.tile([128, N], F16, tag=f"q{m}", name=f"q{m}") for m in range(4)]
    KT = [qkvpool.tile([128, N], F16, tag=f"k{m}", name=f"k{m}") for m in range(4)]
    VT = [qkvpool.tile([128, N], F16, tag=f"v{m}", name=f"v{m}") for m in range(4)]
    APP = [qkvpool.tile([128, N], F16, tag=f"app{m}", name=f"app{m}") for m in range(4)]

    def project(col, m, dst):
        ps = sps.tile([128, N], FP32, tag="s")
        for ic in range(2):
            for kc in range(2):
                nc.tensor.matmul(
                    ps[:, ic * 512 : (ic + 1) * 512],
                    WQC[col][kc][:, m * 128 : (m + 1) * 128],
                    XT[kc][ic][:],
                    start=(kc == 0),
                    stop=(kc == 1),
                )
        with nc.allow_low_precision(reason="f16 activations"):
            nc.vector.tensor_copy(dst[:], ps[:])

    QKV = (QT, KT, VT)
    project(0, 0, QT[0])
    project(1, 0, KT[0])
    project(2, 0, VT[0])
    for t in range(4):
        aw = awps.tile([128, N], FP32, tag="aw")
        z = zpool.tile([128, 16], FP32, tag="z")
        pending = []  # (rhs_a, rhs_b, jt, b_is_f32r); depth LAG

        def colsum(ga, gb, rza, rzb, jt, b32):
            for ic in range(2):
                icsl = slice(ic * 512, (ic + 1) * 512)
                nc.tensor.matmul(
                    aw[0:64, icsl], _bcast64(rza), ga[:, icsl],
                    start=(jt == 0), stop=(jt == 7),
                    tile_position=(0, 0), skip_group_check=True,
                )
                if b32:
                    # odd bf16 halfwords of the int32 Schraudolph tile = the
                    # bf16 rendering of the same exp values (little-endian)
                    gap = gb[:]
                    rhs_b = bass.AP(
                        gap.tensor, gap.offset + ic * 1024 + 1,
                        [list(gap.ap[0]), [2, 512]],
                    )
                else:
                    rhs_b = gb[:, icsl]
                nc.tensor.matmul(
                    aw[64:128, icsl], _bcast64(rzb), rhs_b,
                    start=(jt == 0), stop=(jt == 7),
                    tile_position=(0, 64), skip_group_check=True,
                )

        def recip(jt, b32):
            if b32:
                rza = rzpool.tile([128, 1], F16, tag="rz1")
                with nc.allow_low_precision(reason="f16 matmul lhsT"):
                    nc.vector.reciprocal(rza[:], z[:, 2 * jt : 2 * jt + 1])
                rzb = rz32pool.tile([128, 1], BF16, tag="rzb")
                with nc.allow_low_precision(reason="bf16 matmul lhsT"):
                    nc.vector.reciprocal(rzb[:], z[:, 2 * jt + 1 : 2 * jt + 2])
                return rza[:, 0:1], rzb[:, 0:1]
            rz = rzpool.tile([128, 2], F16, tag="rz")
            with nc.allow_low_precision(reason="f16 matmul lhsT"):
                nc.vector.reciprocal(rz[:], z[:, 2 * jt : 2 * jt + 2])
            return rz[:, 0:1], rz[:, 1:2]

        for jt in range(8):
            jsl = slice(jt * 128, (jt + 1) * 128)
            sa = sps.tile([128, N], FP32, tag="s")
            sb_ = sps.tile([128, N], FP32, tag="s")
            for ic in range(2):
                icsl = slice(ic * 512, (ic + 1) * 512)
                nc.tensor.matmul(
                    sa[:, icsl], KT[t][0:64, jsl], QT[t][0:64, icsl],
                    start=True, stop=True,
                )
            for ic in range(2):
                icsl = slice(ic * 512, (ic + 1) * 512)
                nc.tensor.matmul(
                    sb_[:, icsl], KT[t][64:128, jsl], QT[t][64:128, icsl],
                    start=True, stop=True, tile_position=(64, 0),
                )
            sch = jt in SCH_JTS
            ga = gpool.tile([128, N], F16, tag="g")
            nc.scalar.activation(
                ga[:], sa[:], EXP, scale=1.0 / A0,
                accum_out=z[:, 2 * jt : 2 * jt + 1],
            )
            if sch:
                g32 = g32pool.tile([128, 2 * N], BF16, tag="g32")
                with nc.allow_low_precision(reason="schraudolph exp"):
                    nc.vector.tensor_scalar_add(g32[:].bitcast(I32), sb_[:], B_SCH)
                    nc.vector.reduce_sum(
                        z[:, 2 * jt + 1 : 2 * jt + 2], g32[:].bitcast(FP32),
                        mybir.AxisListType.X,
                    )
                gb = g32
            else:
                gb = gpool.tile([128, N], F16, tag="g")
                nc.scalar.activation(
                    gb[:], sb_[:], EXP, scale=1.0 / A0,
                    accum_out=z[:, 2 * jt + 1 : 2 * jt + 2],
                )
            pending.append((ga, gb, jt, sch))
            if len(pending) > LAG:
                pga, pgb, pjt, psch = pending.pop(0)
                rza, rzb = recip(pjt, psch)
                colsum(pga, pgb, rza, rzb, pjt, psch)
            if t < 3 and jt >= 5:
                col = jt - 5
                project(col, t + 1, QKV[col][t + 1])
        for pga, pgb, pjt, psch in pending:
            rza, rzb = recip(pjt, psch)
            colsum(pga, pgb, rza, rzb, pjt, psch)
        with nc.allow_low_precision(reason="f16 activations"):
            nc.vector.tensor_mul(APP[t][:], VT[t][:], aw[:])

    # Output projection in two 4-chunk waves through the aw PSUM slot.
    for w in range(2):
        po = awps.tile([128, N], FP32, tag="aw")
        for sub in range(4):
            it = w * 4 + sub
            itsl = slice(it * 128, (it + 1) * 128)
            for kt in range(4):
                nc.tensor.matmul(
                    po[:, sub * O : (sub + 1) * O], APP[kt][:, itsl], WOT[kt][:],
                    start=(kt == 0), stop=(kt == 3),
                )
        ob = obpool.tile([128, N], FP32, tag="ob")
        nc.vector.tensor_copy(ob[:], po[:])
        dst = bass.AP(
            out.tensor,
            out.offset + (w * 512) * O,
            [[O, 128], [128 * O, 4], [1, O]],
        )
        nc.sync.dma_start(dst, ob[:])


def build_nc(loop=0, unroll=4, use_bacc=False):
    """loop=0: single body (the graded kernel). loop=L: L body executions
    for timing (For_i trips x unroll bodies + straight-line remainder)."""
    cls = bacc.Bacc if use_bacc else bass.Bass
    nc = cls("TRN2", target_bir_lowering=False, debug=False, num_devices=N_CORES)
    xt = nc.declare_dram_parameter("xt", [C, N], F16, isOutput=False)
    wqkv = nc.declare_dram_parameter("wqkv", [C, 3 * HD], F16, isOutput=False)
    wot = nc.declare_dram_parameter("wot", [HD, O], F16, isOutput=False)
    out = nc.declare_dram_parameter("out", [N, O], FP32, isOutput=True)
    with _TC(nc, num_cores=N_CORES) as tc:
        with ExitStack() as ctx:
            pools = (
                ctx.enter_context(tc.tile_pool(name="w", bufs=2)),
                ctx.enter_context(tc.tile_pool(name="qkv", bufs=2)),
                ctx.enter_context(tc.tile_pool(name="g", bufs=8)),
                ctx.enter_context(tc.tile_pool(name="g32", bufs=4)),
                ctx.enter_context(tc.tile_pool(name="rz", bufs=8)),
                ctx.enter_context(tc.tile_pool(name="rz32", bufs=4)),
                ctx.enter_context(tc.tile_pool(name="z", bufs=2)),
                ctx.enter_context(tc.tile_pool(name="ob", bufs=2)),
                ctx.enter_context(tc.tile_pool(name="sps", bufs=3, space="PSUM")),
                ctx.enter_context(tc.tile_pool(name="awps", bufs=1, space="PSUM")),
            )
            args = (tc, pools, xt.ap(), wqkv.ap(), wot.ap(), out.ap())
            trips, rem = divmod(loop, unroll)
            if loop == 0:
                _emit_body(*args)
            else:
                if trips:
                    with tc.For_i(0, trips, 1):
                        for _ in range(unroll):
                            _emit_body(*args)
                for _ in range(rem):
                    _emit_body(*args)
    return nc


def make_in_maps(features, weight_q, weight_k, weight_v, weight_out):
    wqkv = np.ascontiguousarray(
        np.concatenate(
            [
                (weight_q * C_QK).reshape(C, HD),
                (weight_k * C_QK).reshape(C, HD),
                weight_v.reshape(C, HD),
            ],
            axis=1,
        ),
        dtype=np.float16,
    )
    wot = np.ascontiguousarray(weight_out.reshape(O, HD).T, dtype=np.float16)
    in_maps = []
    for b in range(B):
        xt = np.ascontiguousarray(features[b].T, dtype=np.float16)
        in_maps.append({"xt": xt, "wqkv": wqkv, "wot": wot})
    return in_maps


_CACHED_NC = None


def kernel(features, weight_q, weight_k, weight_v, weight_out):
    global _CACHED_NC
    if _CACHED_NC is None:
        _CACHED_NC = build_nc(loop=0)
    in_maps = make_in_maps(
        np.asarray(features, np.float32),
        np.asarray(weight_q, np.float32),
        np.asarray(weight_k, np.float32),
        np.asarray(weight_v, np.float32),
        np.asarray(weight_out, np.float32),
    )
    res = run_bass_kernel_spmd(_CACHED_NC, in_maps, list(range(N_CORES)))
    return np.stack([res.results[b]["out"] for b in range(B)], axis=0)


if __name__ == "__main__":
    rng = np.random.default_rng(0)
    feats = rng.standard_normal((B, N, C)).astype(np.float32)
    wq = rng.standard_normal((C, H, D)).astype(np.float32) * 0.05
    wk = rng.standard_normal((C, H, D)).astype(np.float32) * 0.05
    wv = rng.standard_normal((C, H, D)).astype(np.float32) * 0.05
    wo = rng.standard_normal((O, H, D)).astype(np.float32) * 0.05
    o = kernel(feats, wq, wk, wv, wo)
    print("kernel ran, out shape", o.shape, "finite:", np.isfinite(o).all())


# revision 3
# speedup vs baseline: 1.0410x; 1.0410x over previous
"""Trainium2 Bass kernel for nn_MultiHeadAttention_62371515073076.

Math (per batch b, faithful to the reference's softmax over the QUERY axis):
  q/k/v = einsum('nc,chd->nhd', x, W{q,k,v})
  s[i,j,h] = q[i,h,:].k[j,h,:] / 8
  attnw[i,h] = sum_j exp(s[i,j,h]) / Z[j,h],  Z[j,h] = sum_i exp(s[i,j,h])
  out = einsum('ihd,ohd->io', v * attnw, Wout)

Sharding: batch 8 -> one batch per NeuronCore (data parallel), weights
replicated.

Engine strategy (HW-microbenched costs per [128,1024] tile):
  - ACT direct-PSUM exp with fused Z row-sum (accum_out): 1631 ns — one
    instruction drains PSUM, exponentiates, and accumulates Z. 48 tiles.
  - DVE "Schraudolph" exp for the other 16 tiles: Q,K are pre-scaled by
    sqrt(A0/8) on the host so PSUM scores hold A0*s; one tensor_scalar_add
    (+B, int32 output) IS exp(s) in float bit-pattern (max rel err 3.9%,
    only 16/64 of the j-contributions -> ~0.5% on attnw). Z for those via
    DVE reduce_sum of the bitcast tile. Zero ACT cost.
  This balances ACT ~78us vs DVE ~76us (vs 105us for the all-ACT or
  bounce-based schedules).
  - attnw accumulated over j by PE: lhsT = (1/Z) broadcast via a stride-0
    AP, two heads col-packed, PSUM-accumulated across the 8 j-tiles.
    Schraudolph halves run this matmul in f32r (dtype must match rhs).
  - Output projection in two 4-chunk waves through the attnw PSUM slot.
"""
import numpy as np
from contextlib import ExitStack

import concourse.bass as bass
import concourse.mybir as mybir
import concourse.tile as tile
from concourse import bacc
from concourse.vector_clock import ScopedClock
from concourse.bass_utils import run_bass_kernel_spmd
import bass_rust

N_CORES = 8
B, N, C, H, D, O = 8, 1024, 256, 8, 64, 256
HD = H * D  # 512
FP32 = mybir.dt.float32
F32R = mybir.dt.float32r
I32 = mybir.dt.int32
F16 = mybir.dt.float16
BF16 = mybir.dt.bfloat16
EXP = mybir.ActivationFunctionType.Exp

# Schraudolph exp-as-int-add constants. PSUM holds A0*s (via host-side
# sqrt(A0/8) scaling of Wq and Wk); bitcast(int32(A0*s + B)) ~ exp(s).
A0 = float(2 ** 23) / float(np.log(2.0))  # 12102203.16
B_SCH = 1064877216.0                      # tuned: max rel err 3.86%
C_QK = float(np.sqrt(A0 / 8.0))

# (t, jt, half) tiles whose exp runs on DVE via Schraudolph (half 1 only).
SCH_JTS = (1, 3, 5, 7)

# colsum pipeline lag behind scores, in jt steps
LAG = 2

_MAXW = 1  # max sync waits this toolchain's walrus accepts per instruction


class _TC(tile.TileContext):
    """TileContext that splits semaphore waits one-per-instruction.

    The walrus build in this toolchain rejects any instruction carrying more
    than one sync wait; engines execute in order, so excess waits move onto
    same-engine NOPs emitted immediately before the instruction.
    """

    def _commit_instruction(self, inst, lazy_reg_writes: bool = True):
        si = inst.sync_info
        if (
            si is not None
            and si.on_wait
            and len(si.on_wait) > _MAXW
            and inst.engine != mybir.EngineType.Unassigned
        ):
            waits = list(si.on_wait)
            inst.sync_info = bass_rust.SyncInfo(
                on_wait=waits[-_MAXW:], on_update=list(si.on_update or [])
            )
            for i in range(0, len(waits) - _MAXW, _MAXW):
                nop = self.nc.engines[inst.engine].nop(nofuse=True, hint="waitsplit")
                nop.ins.sync_info = bass_rust.SyncInfo(
                    on_wait=waits[i : i + _MAXW], on_update=[]
                )
        return super()._commit_instruction(inst, lazy_reg_writes)

    def _drain_and_barrier(self, tick_clock, wait_clock):
        probe = self.nc.sync.drain()
        wait_clock.add_sem_waits(
            probe.ins, ScopedClock({None: tick_clock.global_clock})
        )
        si = probe.ins.sync_info
        waits = list(si.on_wait or []) if si is not None else []
        if len(waits) > 1:
            probe.ins.sync_info = bass_rust.SyncInfo(
                on_wait=waits[:1], on_update=list(si.on_update or [])
            )
            for i in range(1, len(waits)):
                d = self.nc.sync.drain()
                d.ins.sync_info = bass_rust.SyncInfo(
                    on_wait=waits[i : i + 1], on_update=[]
                )
        self.nc.all_engine_barrier()
        assert self.sems is not None
        popped = self.nc._tile_sem_poison_stack.pop()
        assert popped is self._sem_poison
        self.nc.clear_and_free_semaphores(list(self.sems.allocated().values()))
        self.nc.all_engine_barrier()


def _bcast64(col_ap):
    """[P,1] AP -> [P,64] AP reading the same element 64x (free step 0)."""
    return bass.AP(col_ap.tensor, col_ap.offset, [list(col_ap.ap[0]), [0, 64]])


def _emit_body(tc, pools, xt, wqkv, wot, out):
    nc = tc.nc
    (wpool, qkvpool, gpool, g32pool, rzpool, rz32pool, zpool, obpool,
     sps, awps) = pools

    XT = [[None, None], [None, None]]
    WQC = [[None, None], [None, None], [None, None]]
    WOT = []

    def load_x(kc, ic):
        t = wpool.tile([128, 512], F16, tag=f"xt{kc}{ic}", name=f"xt{kc}{ic}")
        nc.sync.dma_start(
            t[:], xt[kc * 128 : (kc + 1) * 128, ic * 512 : (ic + 1) * 512]
        )
        XT[kc][ic] = t

    def load_w(col, kc):
        w = wpool.tile([128, HD], F16, tag=f"w{col}{kc}", name=f"w{col}{kc}")
        nc.sync.dma_start(
            w[:], wqkv[kc * 128 : (kc + 1) * 128, col * HD : (col + 1) * HD]
        )
        WQC[col][kc] = w

    load_x(0, 0); load_x(1, 0); load_w(0, 0); load_w(0, 1)
    load_x(0, 1); load_x(1, 1); load_w(1, 0); load_w(1, 1)
    load_w(2, 0); load_w(2, 1)
    for kt in range(4):
        w = wpool.tile([128, O], F16, tag=f"wot{kt}", name=f"wot{kt}")
        nc.sync.dma_start(w[:], wot[kt * 128 : (kt + 1) * 128, :])
        WOT.append(w)

    QT = [qkvpool.tile([128, N], F16, tag=f"q{m}", name=f"q{m}") for m in range(4)]
    KT = [qkvpool.tile([128, N], F16, tag=f"k{m}", name=f"k{m}") for m in range(4)]
    VT = [qkvpool.tile([128, N], F16, tag=f"v{m}", name=f"v{m}") for m in range(4)]
    APP = [qkvpool.tile([128, N], F16, tag=f"app{m}", name=f"app{m}") for m in range(4)]

    def project(col, m, dst):
        ps = sps.tile([128, N], FP32, tag="s")
        for ic in range(2):
            for kc in range(2):
                nc.tensor.matmul(
                    ps[:, ic * 512 : (ic + 1) * 512],
                    WQC[col][kc][:, m * 128 : (m + 1) * 128],
                    XT[kc][ic][:],
                    start=(kc == 0),
                    stop=(kc == 1),
                )
        with nc.allow_low_precision(reason="f16 activations"):
            nc.vector.tensor_copy(dst[:], ps[:])

    QKV = (QT, KT, VT)
    project(0, 0, QT[0])
    project(1, 0, KT[0])
    project(2, 0, VT[0])
    for t in range(4):
        aw = awps.tile([128, N], FP32, tag="aw")
        z = zpool.tile([128, 16], FP32, tag="z")
        pending = []  # (rhs_a, rhs_b, jt, b_is_f32r); depth LAG

        def colsum(ga, gb, rza, rzb, jt, b32):
            for ic in range(2):
                icsl = slice(ic * 512, (ic + 1) * 512)
                nc.tensor.matmul(
                    aw[0:64, icsl], _bcast64(rza), ga[:, icsl],
                    start=(jt == 0), stop=(jt == 7),
                    tile_position=(0, 0), skip_group_check=True,
                )
                if b32:
                    # odd bf16 halfwords of the int32 Schraudolph tile = the
                    # bf16 rendering of the same exp values (little-endian)
                    gap = gb[:]
                    rhs_b = bass.AP(
                        gap.tensor, gap.offset + ic * 1024 + 1,
                        [list(gap.ap[0]), [2, 512]],
                    )
                else:
                    rhs_b = gb[:, icsl]
                nc.tensor.matmul(
                    aw[64:128, icsl], _bcast64(rzb), rhs_b,
                    start=(jt == 0), stop=(jt == 7),
                    tile_position=(0, 64), skip_group_check=True,
                )

        def recip(jt, b32):
            if b32:
                rza = rzpool.tile([128, 1], F16, tag="rz1")
                with nc.allow_low_precision(reason="f16 matmul lhsT"):
                    nc.vector.reciprocal(rza[:], z[:, 2 * jt : 2 * jt + 1])
                rzb = rz32pool.tile([128, 1], BF16, tag="rzb")
                with nc.allow_low_precision(reason="bf16 matmul lhsT"):
                    nc.vector.reciprocal(rzb[:], z[:, 2 * jt + 1 : 2 * jt + 2])
                return rza[:, 0:1], rzb[:, 0:1]
            rz = rzpool.tile([128, 2], F16, tag="rz")
            with nc.allow_low_precision(reason="f16 matmul lhsT"):
                nc.vector.reciprocal(rz[:], z[:, 2 * jt : 2 * jt + 2])
            return rz[:, 0:1], rz[:, 1:2]

        for jt in range(8):
            jsl = slice(jt * 128, (jt + 1) * 128)
            sa = sps.tile([128, N], FP32, tag="s")
            sb_ = sps.tile([128, N], FP32, tag="s")
            for ic in range(2):
                icsl = slice(ic * 512, (ic + 1) * 512)
                nc.tensor.matmul(
                    sa[:, icsl], KT[t][0:64, jsl], QT[t][0:64, icsl],
                    start=True, stop=True,
                )
            for ic in range(2):
                icsl = slice(ic * 512, (ic + 1) * 512)
                nc.tensor.matmul(
                    sb_[:, icsl], KT[t][64:128, jsl], QT[t][64:128, icsl],
                    start=True, stop=True, tile_position=(64, 0),
                )
            sch = jt in SCH_JTS
            ga = gpool.tile([128, N], F16, tag="g")
            nc.scalar.activation(
                ga[:], sa[:], EXP, scale=1.0 / A0,
                accum_out=z[:, 2 * jt : 2 * jt + 1],
            )
            if sch:
                g32 = g32pool.tile([128, 2 * N], BF16, tag="g32")
                with nc.allow_low_precision(reason="schraudolph exp"):
                    nc.vector.tensor_scalar_add(g32[:].bitcast(I32), sb_[:], B_SCH)
                    nc.vector.reduce_sum(
                        z[:, 2 * jt + 1 : 2 * jt + 2], g32[:].bitcast(FP32),
                        mybir.AxisListType.X,
                    )
                gb = g32
            else:
                gb = gpool.tile([128, N], F16, tag="g")
                nc.scalar.activation(
                    gb[:], sb_[:], EXP, scale=1.0 / A0,
                    accum_out=z[:, 2 * jt + 1 : 2 * jt + 2],
                )
            pending.append((ga, gb, jt, sch))
            if len(pending) > LAG:
                pga, pgb, pjt, psch = pending.pop(0)
                rza, rzb = recip(pjt, psch)
                colsum(pga, pgb, rza, rzb, pjt, psch)
            if t < 3 and jt >= 5:
                col = jt - 5
                project(col, t + 1, QKV[col][t + 1])
        for pga, pgb, pjt, psch in pending:
            rza, rzb = recip(pjt, psch)
            colsum(pga, pgb, rza, rzb, pjt, psch)
        with nc.allow_low_precision(reason="f16 activations"):
            nc.vector.tensor_mul(APP[t][:], VT[t][:], aw[:])

    # Output projection in two 4-chunk waves through the aw PSUM slot.
    for w in range(2):
        po = awps.tile([128, N], FP32, tag="aw")
        for sub in range(4):
            it = w * 4 + sub
            itsl = slice(it * 128, (it + 1) * 128)
            for kt in range(4):
                nc.tensor.matmul(
                    po[:, sub * O : (sub + 1) * O], APP[kt][:, itsl], WOT[kt][:],
                    start=(kt == 0), stop=(kt == 3),
                )
        ob = obpool.tile([128, N], FP32, tag="ob")
        nc.vector.tensor_copy(ob[:], po[:])
        dst = bass.AP(
            out.tensor,
            out.offset + (w * 512) * O,
            [[O, 128], [128 * O, 4], [1, O]],
        )
        nc.sync.dma_start(dst, ob[:])


def build_nc(loop=0, unroll=4, use_bacc=False):
    """loop=0: single body (the graded kernel). loop=L: L body executions
    for timing (For_i trips x unroll bodies + straight-line remainder)."""
    cls = bacc.Bacc if use_bacc else bass.Bass
    nc = cls("TRN2", target_bir_lowering=False, debug=False, num_devices=N_CORES)
    xt = nc.declare_dram_parameter("xt", [C, N], F16, isOutput=False)
    wqkv = nc.declare_dram_parameter("wqkv", [C, 3 * HD], F16, isOutput=False)
    wot = nc.declare_dram_parameter("wot", [HD, O], F16, isOutput=False)
    out = nc.declare_dram_parameter("out", [N, O], FP32, isOutput=True)
    with _TC(nc, num_cores=N_CORES) as tc:
        with ExitStack() as ctx:
            pools = (
                ctx.enter_context(tc.tile_pool(name="w", bufs=2)),
                ctx.enter_context(tc.tile_pool(name="qkv", bufs=2)),
                ctx.enter_context(tc.tile_pool(name="g", bufs=12)),
                ctx.enter_context(tc.tile_pool(name="g32", bufs=6)),
                ctx.enter_context(tc.tile_pool(name="rz", bufs=8)),
                ctx.enter_context(tc.tile_pool(name="rz32", bufs=4)),
                ctx.enter_context(tc.tile_pool(name="z", bufs=2)),
                ctx.enter_context(tc.tile_pool(name="ob", bufs=2)),
                ctx.enter_context(tc.tile_pool(name="sps", bufs=3, space="PSUM")),
                ctx.enter_context(tc.tile_pool(name="awps", bufs=1, space="PSUM")),
            )
            args = (tc, pools, xt.ap(), wqkv.ap(), wot.ap(), out.ap())
            trips, rem = divmod(loop, unroll)
            if loop == 0:
                _emit_body(*args)
            else:
                if trips:
                    with tc.For_i(0, trips, 1):
                        for _ in range(unroll):
                            _emit_body(*args)
                for _ in range(rem):
                    _emit_body(*args)
    return nc


def make_in_maps(features, weight_q, weight_k, weight_v, weight_out):
    wqkv = np.ascontiguousarray(
        np.concatenate(
            [
                (weight_q * C_QK).reshape(C, HD),
                (weight_k * C_QK).reshape(C, HD),
                weight_v.reshape(C, HD),
            ],
            axis=1,
        ),
        dtype=np.float16,
    )
    wot = np.ascontiguousarray(weight_out.reshape(O, HD).T, dtype=np.float16)
    in_maps = []
    for b in range(B):
        xt = np.ascontiguousarray(features[b].T, dtype=np.float16)
        in_maps.append({"xt": xt, "wqkv": wqkv, "wot": wot})
    return in_maps


_CACHED_NC = None


def kernel(features, weight_q, weight_k, weight_v, weight_out):
    global _CACHED_NC
    if _CACHED_NC is None:
        _CACHED_NC = build_nc(loop=0)
    in_maps = make_in_maps(
        np.asarray(features, np.float32),
        np.asarray(weight_q, np.float32),
        np.asarray(weight_k, np.float32),
        np.asarray(weight_v, np.float32),
        np.asarray(weight_out, np.float32),
    )
    res = run_bass_kernel_spmd(_CACHED_NC, in_maps, list(range(N_CORES)))
    return np.stack([res.results[b]["out"] for b in range(B)], axis=0)


if __name__ == "__main__":
    rng = np.random.default_rng(0)
    feats = rng.standard_normal((B, N, C)).astype(np.float32)
    wq = rng.standard_normal((C, H, D)).astype(np.float32) * 0.05
    wk = rng.standard_normal((C, H, D)).astype(np.float32) * 0.05
    wv = rng.standard_normal((C, H, D)).astype(np.float32) * 0.05
    wo = rng.standard_normal((O, H, D)).astype(np.float32) * 0.05
    o = kernel(feats, wq, wk, wv, wo)
    print("kernel ran, out shape", o.shape, "finite:", np.isfinite(o).all())


# revision 4
# speedup vs baseline: 1.0460x; 1.0048x over previous
"""Trainium2 Bass kernel for nn_MultiHeadAttention_62371515073076.

Math (per batch b, faithful to the reference's softmax over the QUERY axis):
  q/k/v = einsum('nc,chd->nhd', x, W{q,k,v})
  s[i,j,h] = q[i,h,:].k[j,h,:] / 8
  attnw[i,h] = sum_j exp(s[i,j,h]) / Z[j,h],  Z[j,h] = sum_i exp(s[i,j,h])
  out = einsum('ihd,ohd->io', v * attnw, Wout)

Sharding: batch 8 -> one batch per NeuronCore (data parallel), weights
replicated.

Engine strategy (HW-microbenched costs per [128,1024] tile):
  - ACT direct-PSUM exp with fused Z row-sum (accum_out): 1631 ns — one
    instruction drains PSUM, exponentiates, and accumulates Z. 48 tiles.
  - DVE "Schraudolph" exp for the other 16 tiles: Q,K are pre-scaled by
    sqrt(A0/8) on the host so PSUM scores hold A0*s; one tensor_scalar_add
    (+B, int32 output) IS exp(s) in float bit-pattern (max rel err 3.9%,
    only 16/64 of the j-contributions -> ~0.5% on attnw). Z for those via
    DVE reduce_sum of the bitcast tile. Zero ACT cost.
  This balances ACT ~78us vs DVE ~76us (vs 105us for the all-ACT or
  bounce-based schedules).
  - attnw accumulated over j by PE: lhsT = (1/Z) broadcast via a stride-0
    AP, two heads col-packed, PSUM-accumulated across the 8 j-tiles.
    Schraudolph halves run this matmul in f32r (dtype must match rhs).
  - Output projection in two 4-chunk waves through the attnw PSUM slot.
"""
import numpy as np
from contextlib import ExitStack

import concourse.bass as bass
import concourse.mybir as mybir
import concourse.tile as tile
from concourse import bacc
from concourse.vector_clock import ScopedClock
from concourse.bass_utils import run_bass_kernel_spmd
import bass_rust

N_CORES = 8
B, N, C, H, D, O = 8, 1024, 256, 8, 64, 256
HD = H * D  # 512
FP32 = mybir.dt.float32
F32R = mybir.dt.float32r
I32 = mybir.dt.int32
F16 = mybir.dt.float16
BF16 = mybir.dt.bfloat16
EXP = mybir.ActivationFunctionType.Exp

# Schraudolph exp-as-int-add constants. PSUM holds A0*s (via host-side
# sqrt(A0/8) scaling of Wq and Wk); bitcast(int32(A0*s + B)) ~ exp(s).
A0 = float(2 ** 23) / float(np.log(2.0))  # 12102203.16
B_SCH = 1064877216.0                      # tuned: max rel err 3.86%
C_QK = float(np.sqrt(A0 / 8.0))

# (t, jt, half) tiles whose exp runs on DVE via Schraudolph (half 1 only).
SCH_JTS = (1, 3, 5, 7)

# colsum pipeline lag behind scores, in jt steps
LAG = 2

_MAXW = 1  # max sync waits this toolchain's walrus accepts per instruction


class _TC(tile.TileContext):
    """TileContext that splits semaphore waits one-per-instruction.

    The walrus build in this toolchain rejects any instruction carrying more
    than one sync wait; engines execute in order, so excess waits move onto
    same-engine NOPs emitted immediately before the instruction.
    """

    def _commit_instruction(self, inst, lazy_reg_writes: bool = True):
        si = inst.sync_info
        if (
            si is not None
            and si.on_wait
            and len(si.on_wait) > _MAXW
            and inst.engine != mybir.EngineType.Unassigned
        ):
            waits = list(si.on_wait)
            inst.sync_info = bass_rust.SyncInfo(
                on_wait=waits[-_MAXW:], on_update=list(si.on_update or [])
            )
            for i in range(0, len(waits) - _MAXW, _MAXW):
                nop = self.nc.engines[inst.engine].nop(nofuse=True, hint="waitsplit")
                nop.ins.sync_info = bass_rust.SyncInfo(
                    on_wait=waits[i : i + _MAXW], on_update=[]
                )
        return super()._commit_instruction(inst, lazy_reg_writes)

    def _drain_and_barrier(self, tick_clock, wait_clock):
        probe = self.nc.sync.drain()
        wait_clock.add_sem_waits(
            probe.ins, ScopedClock({None: tick_clock.global_clock})
        )
        si = probe.ins.sync_info
        waits = list(si.on_wait or []) if si is not None else []
        if len(waits) > 1:
            probe.ins.sync_info = bass_rust.SyncInfo(
                on_wait=waits[:1], on_update=list(si.on_update or [])
            )
            for i in range(1, len(waits)):
                d = self.nc.sync.drain()
                d.ins.sync_info = bass_rust.SyncInfo(
                    on_wait=waits[i : i + 1], on_update=[]
                )
        self.nc.all_engine_barrier()
        assert self.sems is not None
        popped = self.nc._tile_sem_poison_stack.pop()
        assert popped is self._sem_poison
        self.nc.clear_and_free_semaphores(list(self.sems.allocated().values()))
        self.nc.all_engine_barrier()


def _bcast64(col_ap):
    """[P,1] AP -> [P,64] AP reading the same element 64x (free step 0)."""
    return bass.AP(col_ap.tensor, col_ap.offset, [list(col_ap.ap[0]), [0, 64]])


def _emit_body(tc, pools, xt, wqkv, wot, out):
    nc = tc.nc
    (wpool, qkvpool, gpool, g32pool, rzpool, rz32pool, zpool, obpool,
     sps, awps) = pools

    XT = [[None, None], [None, None]]
    WQC = [[None, None], [None, None], [None, None]]
    WOT = []

    def load_x(kc, ic):
        t = wpool.tile([128, 512], F16, tag=f"xt{kc}{ic}", name=f"xt{kc}{ic}")
        nc.sync.dma_start(
            t[:], xt[kc * 128 : (kc + 1) * 128, ic * 512 : (ic + 1) * 512]
        )
        XT[kc][ic] = t

    def load_w(col, kc):
        w = wpool.tile([128, HD], F16, tag=f"w{col}{kc}", name=f"w{col}{kc}")
        nc.sync.dma_start(
            w[:], wqkv[kc * 128 : (kc + 1) * 128, col * HD : (col + 1) * HD]
        )
        WQC[col][kc] = w

    load_x(0, 0); load_x(1, 0); load_w(0, 0); load_w(0, 1)
    load_x(0, 1); load_x(1, 1); load_w(1, 0); load_w(1, 1)
    load_w(2, 0); load_w(2, 1)
    for kt in range(4):
        w = wpool.tile([128, O], F16, tag=f"wot{kt}", name=f"wot{kt}")
        nc.sync.dma_start(w[:], wot[kt * 128 : (kt + 1) * 128, :])
        WOT.append(w)

    QT = [qkvpool.tile([128, N], F16, tag=f"q{m}", name=f"q{m}") for m in range(4)]
    KT = [qkvpool.tile([128, N], F16, tag=f"k{m}", name=f"k{m}") for m in range(4)]
    VT = [qkvpool.tile([128, N], F16, tag=f"v{m}", name=f"v{m}") for m in range(4)]
    APP = [qkvpool.tile([128, N], F16, tag=f"app{m}", name=f"app{m}") for m in range(4)]

    def project(col, m, dst):
        ps = sps.tile([128, N], FP32, tag="s")
        for ic in range(2):
            for kc in range(2):
                nc.tensor.matmul(
                    ps[:, ic * 512 : (ic + 1) * 512],
                    WQC[col][kc][:, m * 128 : (m + 1) * 128],
                    XT[kc][ic][:],
                    start=(kc == 0),
                    stop=(kc == 1),
                )
        with nc.allow_low_precision(reason="f16 activations"):
            nc.vector.tensor_copy(dst[:], ps[:])

    QKV = (QT, KT, VT)
    project(0, 0, QT[0])
    project(1, 0, KT[0])
    project(2, 0, VT[0])
    for t in range(4):
        aw = awps.tile([128, N], FP32, tag="aw")
        z = zpool.tile([128, 16], FP32, tag="z")
        pending = []  # (rhs_a, rhs_b, jt, b_is_f32r); depth LAG

        def colsum(ga, gb, rza, rzb, jt, b32):
            for ic in range(2):
                icsl = slice(ic * 512, (ic + 1) * 512)
                nc.tensor.matmul(
                    aw[0:64, icsl], _bcast64(rza), ga[:, icsl],
                    start=(jt == 0), stop=(jt == 7),
                    tile_position=(0, 0), skip_group_check=True,
                )
                if b32:
                    # odd bf16 halfwords of the int32 Schraudolph tile = the
                    # bf16 rendering of the same exp values (little-endian)
                    gap = gb[:]
                    rhs_b = bass.AP(
                        gap.tensor, gap.offset + ic * 1024 + 1,
                        [list(gap.ap[0]), [2, 512]],
                    )
                else:
                    rhs_b = gb[:, icsl]
                nc.tensor.matmul(
                    aw[64:128, icsl], _bcast64(rzb), rhs_b,
                    start=(jt == 0), stop=(jt == 7),
                    tile_position=(0, 64), skip_group_check=True,
                )

        def recip(jt, b32):
            if b32:
                rza = rzpool.tile([128, 1], F16, tag="rz1")
                with nc.allow_low_precision(reason="f16 matmul lhsT"):
                    nc.vector.reciprocal(rza[:], z[:, 2 * jt : 2 * jt + 1])
                rzb = rz32pool.tile([128, 1], BF16, tag="rzb")
                with nc.allow_low_precision(reason="bf16 matmul lhsT"):
                    nc.vector.reciprocal(rzb[:], z[:, 2 * jt + 1 : 2 * jt + 2])
                return rza[:, 0:1], rzb[:, 0:1]
            rz = rzpool.tile([128, 2], F16, tag="rz")
            with nc.allow_low_precision(reason="f16 matmul lhsT"):
                nc.vector.reciprocal(rz[:], z[:, 2 * jt : 2 * jt + 2])
            return rz[:, 0:1], rz[:, 1:2]

        for jt in range(8):
            jsl = slice(jt * 128, (jt + 1) * 128)
            sa = sps.tile([128, N], FP32, tag="s")
            sb_ = sps.tile([128, N], FP32, tag="s")
            for ic in range(2):
                icsl = slice(ic * 512, (ic + 1) * 512)
                nc.tensor.matmul(
                    sa[:, icsl], KT[t][0:64, jsl], QT[t][0:64, icsl],
                    start=True, stop=True,
                )
            for ic in range(2):
                icsl = slice(ic * 512, (ic + 1) * 512)
                nc.tensor.matmul(
                    sb_[:, icsl], KT[t][64:128, jsl], QT[t][64:128, icsl],
                    start=True, stop=True, tile_position=(64, 0),
                )
            sch = jt in SCH_JTS
            ga = gpool.tile([128, N], F16, tag="g")
            nc.scalar.activation(
                ga[:], sa[:], EXP, scale=1.0 / A0,
                accum_out=z[:, 2 * jt : 2 * jt + 1],
            )
            if sch:
                g32 = g32pool.tile([128, 2 * N], BF16, tag="g32")
                with nc.allow_low_precision(reason="schraudolph exp"):
                    nc.vector.tensor_scalar_add(g32[:].bitcast(I32), sb_[:], B_SCH)
                    nc.vector.reduce_sum(
                        z[:, 2 * jt + 1 : 2 * jt + 2], g32[:].bitcast(FP32),
                        mybir.AxisListType.X,
                    )
                gb = g32
            else:
                gb = gpool.tile([128, N], F16, tag="g")
                nc.scalar.activation(
                    gb[:], sb_[:], EXP, scale=1.0 / A0,
                    accum_out=z[:, 2 * jt + 1 : 2 * jt + 2],
                )
            pending.append((ga, gb, jt, sch))
            if len(pending) > LAG:
                pga, pgb, pjt, psch = pending.pop(0)
                rza, rzb = recip(pjt, psch)
                colsum(pga, pgb, rza, rzb, pjt, psch)
            if t < 3 and jt in (0, 2, 4):
                # even jts carry no Schraudolph conv/reduce -- landing the
                # projection drains here smooths the per-step DVE load
                col = jt // 2
                project(col, t + 1, QKV[col][t + 1])
        for pga, pgb, pjt, psch in pending:
            rza, rzb = recip(pjt, psch)
            colsum(pga, pgb, rza, rzb, pjt, psch)
        with nc.allow_low_precision(reason="f16 activations"):
            nc.vector.tensor_mul(APP[t][:], VT[t][:], aw[:])

    # Output projection in two 4-chunk waves through the aw PSUM slot.
    for w in range(2):
        po = awps.tile([128, N], FP32, tag="aw")
        for sub in range(4):
            it = w * 4 + sub
            itsl = slice(it * 128, (it + 1) * 128)
            for kt in range(4):
                nc.tensor.matmul(
                    po[:, sub * O : (sub + 1) * O], APP[kt][:, itsl], WOT[kt][:],
                    start=(kt == 0), stop=(kt == 3),
                )
        ob = obpool.tile([128, N], FP32, tag="ob")
        nc.vector.tensor_copy(ob[:], po[:])
        dst = bass.AP(
            out.tensor,
            out.offset + (w * 512) * O,
            [[O, 128], [128 * O, 4], [1, O]],
        )
        nc.sync.dma_start(dst, ob[:])


def build_nc(loop=0, unroll=4, use_bacc=False):
    """loop=0: single body (the graded kernel). loop=L: L body executions
    for timing (For_i trips x unroll bodies + straight-line remainder)."""
    cls = bacc.Bacc if use_bacc else bass.Bass
    nc = cls("TRN2", target_bir_lowering=False, debug=False, num_devices=N_CORES)
    xt = nc.declare_dram_parameter("xt", [C, N], F16, isOutput=False)
    wqkv = nc.declare_dram_parameter("wqkv", [C, 3 * HD], F16, isOutput=False)
    wot = nc.declare_dram_parameter("wot", [HD, O], F16, isOutput=False)
    out = nc.declare_dram_parameter("out", [N, O], FP32, isOutput=True)
    with _TC(nc, num_cores=N_CORES) as tc:
        with ExitStack() as ctx:
            pools = (
                ctx.enter_context(tc.tile_pool(name="w", bufs=2)),
                ctx.enter_context(tc.tile_pool(name="qkv", bufs=2)),
                ctx.enter_context(tc.tile_pool(name="g", bufs=12)),
                ctx.enter_context(tc.tile_pool(name="g32", bufs=6)),
                ctx.enter_context(tc.tile_pool(name="rz", bufs=8)),
                ctx.enter_context(tc.tile_pool(name="rz32", bufs=4)),
                ctx.enter_context(tc.tile_pool(name="z", bufs=2)),
                ctx.enter_context(tc.tile_pool(name="ob", bufs=2)),
                ctx.enter_context(tc.tile_pool(name="sps", bufs=3, space="PSUM")),
                ctx.enter_context(tc.tile_pool(name="awps", bufs=1, space="PSUM")),
            )
            args = (tc, pools, xt.ap(), wqkv.ap(), wot.ap(), out.ap())
            trips, rem = divmod(loop, unroll)
            if loop == 0:
                _emit_body(*args)
            else:
                if trips:
                    with tc.For_i(0, trips, 1):
                        for _ in range(unroll):
                            _emit_body(*args)
                for _ in range(rem):
                    _emit_body(*args)
    return nc


def make_in_maps(features, weight_q, weight_k, weight_v, weight_out):
    wqkv = np.ascontiguousarray(
        np.concatenate(
            [
                (weight_q * C_QK).reshape(C, HD),
                (weight_k * C_QK).reshape(C, HD),
                weight_v.reshape(C, HD),
            ],
            axis=1,
        ),
        dtype=np.float16,
    )
    wot = np.ascontiguousarray(weight_out.reshape(O, HD).T, dtype=np.float16)
    in_maps = []
    for b in range(B):
        xt = np.ascontiguousarray(features[b].T, dtype=np.float16)
        in_maps.append({"xt": xt, "wqkv": wqkv, "wot": wot})
    return in_maps


_CACHED_NC = None


def kernel(features, weight_q, weight_k, weight_v, weight_out):
    global _CACHED_NC
    if _CACHED_NC is None:
        _CACHED_NC = build_nc(loop=0)
    in_maps = make_in_maps(
        np.asarray(features, np.float32),
        np.asarray(weight_q, np.float32),
        np.asarray(weight_k, np.float32),
        np.asarray(weight_v, np.float32),
        np.asarray(weight_out, np.float32),
    )
    res = run_bass_kernel_spmd(_CACHED_NC, in_maps, list(range(N_CORES)))
    return np.stack([res.results[b]["out"] for b in range(B)], axis=0)


if __name__ == "__main__":
    rng = np.random.default_rng(0)
    feats = rng.standard_normal((B, N, C)).astype(np.float32)
    wq = rng.standard_normal((C, H, D)).astype(np.float32) * 0.05
    wk = rng.standard_normal((C, H, D)).astype(np.float32) * 0.05
    wv = rng.standard_normal((C, H, D)).astype(np.float32) * 0.05
    wo = rng.standard_normal((O, H, D)).astype(np.float32) * 0.05
    o = kernel(feats, wq, wk, wv, wo)
    print("kernel ran, out shape", o.shape, "finite:", np.isfinite(o).all())


# revision 6
# speedup vs baseline: 1.0565x; 1.0100x over previous
"""Trainium2 Bass kernel for nn_MultiHeadAttention_62371515073076.

Math (per batch b, faithful to the reference's softmax over the QUERY axis):
  q/k/v = einsum('nc,chd->nhd', x, W{q,k,v})
  s[i,j,h] = q[i,h,:].k[j,h,:] / 8
  attnw[i,h] = sum_j exp(s[i,j,h]) / Z[j,h],  Z[j,h] = sum_i exp(s[i,j,h])
  out = einsum('ihd,ohd->io', v * attnw, Wout)

Sharding: batch 8 -> one batch per NeuronCore (data parallel), weights
replicated.

Engine strategy (HW-microbenched costs per [128,1024] tile):
  - ACT direct-PSUM exp with fused Z row-sum (accum_out): 1631 ns — one
    instruction drains PSUM, exponentiates, and accumulates Z. 48 tiles.
  - DVE "Schraudolph" exp for the other 16 tiles: Q,K are pre-scaled by
    sqrt(A0/8) on the host so PSUM scores hold A0*s; one tensor_scalar_add
    (+B, int32 output) IS exp(s) in float bit-pattern (max rel err 3.9%,
    only 16/64 of the j-contributions -> ~0.5% on attnw). Z for those via
    DVE reduce_sum of the bitcast tile. Zero ACT cost.
  This balances ACT ~78us vs DVE ~76us (vs 105us for the all-ACT or
  bounce-based schedules).
  - attnw accumulated over j by PE: lhsT = (1/Z) broadcast via a stride-0
    AP, two heads col-packed, PSUM-accumulated across the 8 j-tiles.
    Schraudolph halves run this matmul in f32r (dtype must match rhs).
  - Output projection in two 4-chunk waves through the attnw PSUM slot.
"""
import numpy as np
from contextlib import ExitStack

import concourse.bass as bass
import concourse.mybir as mybir
import concourse.tile as tile
from concourse import bacc
from concourse.vector_clock import ScopedClock
from concourse.bass_utils import run_bass_kernel_spmd
import bass_rust

N_CORES = 8
B, N, C, H, D, O = 8, 1024, 256, 8, 64, 256
HD = H * D  # 512
FP32 = mybir.dt.float32
F32R = mybir.dt.float32r
I32 = mybir.dt.int32
F16 = mybir.dt.float16
BF16 = mybir.dt.bfloat16
EXP = mybir.ActivationFunctionType.Exp

# Schraudolph exp-as-int-add constants. PSUM holds A0*s (via host-side
# sqrt(A0/8) scaling of Wq and Wk); bitcast(int32(A0*s + B)) ~ exp(s).
A0 = float(2 ** 23) / float(np.log(2.0))  # 12102203.16
B_SCH = 1064877216.0                      # tuned: max rel err 3.86%
C_QK = float(np.sqrt(A0 / 8.0))

# (t, jt, half) tiles whose exp runs on DVE via Schraudolph (half 1 only).
SCH_JTS = (1, 3, 5, 7)

# colsum pipeline lag behind scores, in jt steps
LAG = 2

_MAXW = 1  # max sync waits this toolchain's walrus accepts per instruction


class _TC(tile.TileContext):
    """TileContext that splits semaphore waits one-per-instruction.

    The walrus build in this toolchain rejects any instruction carrying more
    than one sync wait; engines execute in order, so excess waits move onto
    same-engine NOPs emitted immediately before the instruction.
    """

    def _commit_instruction(self, inst, lazy_reg_writes: bool = True):
        si = inst.sync_info
        if (
            si is not None
            and si.on_wait
            and len(si.on_wait) > _MAXW
            and inst.engine != mybir.EngineType.Unassigned
        ):
            waits = list(si.on_wait)
            inst.sync_info = bass_rust.SyncInfo(
                on_wait=waits[-_MAXW:], on_update=list(si.on_update or [])
            )
            for i in range(0, len(waits) - _MAXW, _MAXW):
                nop = self.nc.engines[inst.engine].nop(nofuse=True, hint="waitsplit")
                nop.ins.sync_info = bass_rust.SyncInfo(
                    on_wait=waits[i : i + _MAXW], on_update=[]
                )
        return super()._commit_instruction(inst, lazy_reg_writes)

    def _drain_and_barrier(self, tick_clock, wait_clock):
        probe = self.nc.sync.drain()
        wait_clock.add_sem_waits(
            probe.ins, ScopedClock({None: tick_clock.global_clock})
        )
        si = probe.ins.sync_info
        waits = list(si.on_wait or []) if si is not None else []
        if len(waits) > 1:
            probe.ins.sync_info = bass_rust.SyncInfo(
                on_wait=waits[:1], on_update=list(si.on_update or [])
            )
            for i in range(1, len(waits)):
                d = self.nc.sync.drain()
                d.ins.sync_info = bass_rust.SyncInfo(
                    on_wait=waits[i : i + 1], on_update=[]
                )
        self.nc.all_engine_barrier()
        assert self.sems is not None
        popped = self.nc._tile_sem_poison_stack.pop()
        assert popped is self._sem_poison
        self.nc.clear_and_free_semaphores(list(self.sems.allocated().values()))
        self.nc.all_engine_barrier()


def _bcast64(col_ap):
    """[P,1] AP -> [P,64] AP reading the same element 64x (free step 0)."""
    return bass.AP(col_ap.tensor, col_ap.offset, [list(col_ap.ap[0]), [0, 64]])


def _emit_body(tc, pools, xt, wqkv, wot, out):
    nc = tc.nc
    (wpool, qkvpool, gpool, g32pool, rzpool, rz32pool, zpool, obpool,
     sps, awps) = pools

    XT = [[None, None], [None, None]]
    WQC = [[None, None], [None, None], [None, None]]
    WOT = []

    def load_x(kc, ic):
        # one [128,1024] tile per kc (half the DMAs/semaphores); ic selects
        # the 512-wide slice view
        if XT[kc][0] is None and XT[kc][1] is None:
            t = wpool.tile([128, 1024], F16, tag=f"xt{kc}", name=f"xt{kc}")
            nc.sync.dma_start(t[:], xt[kc * 128 : (kc + 1) * 128, :])
            XT[kc][0] = t[:, 0:512]
            XT[kc][1] = t[:, 512:1024]

    def load_w(col, kc):
        w = wpool.tile([128, HD], F16, tag=f"w{col}{kc}", name=f"w{col}{kc}")
        nc.sync.dma_start(
            w[:], wqkv[kc * 128 : (kc + 1) * 128, col * HD : (col + 1) * HD]
        )
        WQC[col][kc] = w

    load_x(0, 0); load_x(1, 0); load_w(0, 0); load_w(0, 1)
    load_x(0, 1); load_x(1, 1); load_w(1, 0); load_w(1, 1)
    load_w(2, 0); load_w(2, 1)
    for kt in range(4):
        w = wpool.tile([128, O], F16, tag=f"wot{kt}", name=f"wot{kt}")
        nc.sync.dma_start(w[:], wot[kt * 128 : (kt + 1) * 128, :])
        WOT.append(w)

    QT = [qkvpool.tile([128, N], F16, tag=f"q{m}", name=f"q{m}") for m in range(4)]
    KT = [qkvpool.tile([128, N], F16, tag=f"k{m}", name=f"k{m}") for m in range(4)]
    VT = [qkvpool.tile([128, N], F16, tag=f"v{m}", name=f"v{m}") for m in range(4)]
    APP = [qkvpool.tile([128, N], F16, tag=f"app{m}", name=f"app{m}") for m in range(4)]

    def project(col, m, dst):
        ps = sps.tile([128, N], FP32, tag="s")
        for ic in range(2):
            for kc in range(2):
                nc.tensor.matmul(
                    ps[:, ic * 512 : (ic + 1) * 512],
                    WQC[col][kc][:, m * 128 : (m + 1) * 128],
                    XT[kc][ic],
                    start=(kc == 0),
                    stop=(kc == 1),
                )
        with nc.allow_low_precision(reason="f16 activations"):
            nc.vector.tensor_copy(dst[:], ps[:])

    QKV = (QT, KT, VT)
    project(0, 0, QT[0])
    project(1, 0, KT[0])
    project(2, 0, VT[0])
    carry = None  # deferred flush (colsums+APP) of the previous t
    for t in range(4):
        aw = awps.tile([128, N], FP32, tag="aw")
        z = zpool.tile([128, 16], FP32, tag="z")
        pending = []  # (rhs_a, rhs_b, jt, b_is_f32r); depth LAG

        def colsum(ga, gb, rza, rzb, jt, b32, aw=aw):
            for ic in range(2):
                icsl = slice(ic * 512, (ic + 1) * 512)
                nc.tensor.matmul(
                    aw[0:64, icsl], _bcast64(rza), ga[:, icsl],
                    start=(jt == 0), stop=(jt == 7),
                    tile_position=(0, 0), skip_group_check=True,
                )
                if b32:
                    # odd bf16 halfwords of the int32 Schraudolph tile = the
                    # bf16 rendering of the same exp values (little-endian)
                    gap = gb[:]
                    rhs_b = bass.AP(
                        gap.tensor, gap.offset + ic * 1024 + 1,
                        [list(gap.ap[0]), [2, 512]],
                    )
                else:
                    rhs_b = gb[:, icsl]
                nc.tensor.matmul(
                    aw[64:128, icsl], _bcast64(rzb), rhs_b,
                    start=(jt == 0), stop=(jt == 7),
                    tile_position=(0, 64), skip_group_check=True,
                )

        def recip(jt, b32, z=z):
            if b32:
                rza = rzpool.tile([128, 1], F16, tag="rz1")
                with nc.allow_low_precision(reason="f16 matmul lhsT"):
                    nc.vector.reciprocal(rza[:], z[:, 2 * jt : 2 * jt + 1])
                rzb = rz32pool.tile([128, 1], BF16, tag="rzb")
                with nc.allow_low_precision(reason="bf16 matmul lhsT"):
                    nc.vector.reciprocal(rzb[:], z[:, 2 * jt + 1 : 2 * jt + 2])
                return rza[:, 0:1], rzb[:, 0:1]
            rz = rzpool.tile([128, 2], F16, tag="rz")
            with nc.allow_low_precision(reason="f16 matmul lhsT"):
                nc.vector.reciprocal(rz[:], z[:, 2 * jt : 2 * jt + 2])
            return rz[:, 0:1], rz[:, 1:2]

        for jt in range(8):
            jsl = slice(jt * 128, (jt + 1) * 128)
            sa = sps.tile([128, N], FP32, tag="s")
            sb_ = sps.tile([128, N], FP32, tag="s")
            for ic in range(2):
                icsl = slice(ic * 512, (ic + 1) * 512)
                nc.tensor.matmul(
                    sa[:, icsl], KT[t][0:64, jsl], QT[t][0:64, icsl],
                    start=True, stop=True,
                )
            for ic in range(2):
                icsl = slice(ic * 512, (ic + 1) * 512)
                nc.tensor.matmul(
                    sb_[:, icsl], KT[t][64:128, jsl], QT[t][64:128, icsl],
                    start=True, stop=True, tile_position=(64, 0),
                )
            # emit the previous t's flush AFTER this step's score matmuls:
            # PE is in-order, so this puts t's first scores ahead of the 12
            # flush colsums and lets ACT start t's exps ~2us earlier
            if jt == 0 and carry is not None:
                carry()
                carry = None
            sch = jt in SCH_JTS
            ga = gpool.tile([128, N], F16, tag="g")
            nc.scalar.activation(
                ga[:], sa[:], EXP, scale=1.0 / A0,
                accum_out=z[:, 2 * jt : 2 * jt + 1],
            )
            if sch:
                g32 = g32pool.tile([128, 2 * N], BF16, tag="g32")
                with nc.allow_low_precision(reason="schraudolph exp"):
                    nc.vector.tensor_scalar_add(g32[:].bitcast(I32), sb_[:], B_SCH)
                    nc.vector.reduce_sum(
                        z[:, 2 * jt + 1 : 2 * jt + 2], g32[:].bitcast(FP32),
                        mybir.AxisListType.X,
                    )
                gb = g32
            else:
                gb = gpool.tile([128, N], F16, tag="g")
                nc.scalar.activation(
                    gb[:], sb_[:], EXP, scale=1.0 / A0,
                    accum_out=z[:, 2 * jt + 1 : 2 * jt + 2],
                )
            pending.append((ga, gb, jt, sch))
            if len(pending) > LAG:
                pga, pgb, pjt, psch = pending.pop(0)
                rza, rzb = recip(pjt, psch)
                colsum(pga, pgb, rza, rzb, pjt, psch)
            if t < 3 and jt in (0, 2, 4):
                # even jts carry no Schraudolph conv/reduce -- landing the
                # projection drains here smooths the per-step DVE load
                col = jt // 2
                project(col, t + 1, QKV[col][t + 1])
        def _flush(pending=pending, recip=recip, colsum=colsum, t=t, aw=aw):
            for pga, pgb, pjt, psch in pending:
                rza, rzb = recip(pjt, psch)
                colsum(pga, pgb, rza, rzb, pjt, psch)
            with nc.allow_low_precision(reason="f16 activations"):
                nc.vector.tensor_mul(APP[t][:, 0:512], VT[t][:, 0:512],
                                     aw[:, 0:512])
                nc.vector.tensor_mul(APP[t][:, 512:1024], VT[t][:, 512:1024],
                                     aw[:, 512:1024])

        carry = _flush
    carry()

    # Output projection in two 4-chunk waves through the aw PSUM slot.
    for w in range(2):
        po = awps.tile([128, N], FP32, tag="aw")
        for sub in range(4):
            it = w * 4 + sub
            itsl = slice(it * 128, (it + 1) * 128)
            for kt in range(4):
                nc.tensor.matmul(
                    po[:, sub * O : (sub + 1) * O], APP[kt][:, itsl], WOT[kt][:],
                    start=(kt == 0), stop=(kt == 3),
                )
        ob = obpool.tile([128, N], FP32, tag="ob")
        nc.vector.tensor_copy(ob[:], po[:])
        dst = bass.AP(
            out.tensor,
            out.offset + (w * 512) * O,
            [[O, 128], [128 * O, 4], [1, O]],
        )
        nc.sync.dma_start(dst, ob[:])


def build_nc(loop=0, unroll=4, use_bacc=False):
    """loop=0: single body (the graded kernel). loop=L: L body executions
    for timing (For_i trips x unroll bodies + straight-line remainder)."""
    cls = bacc.Bacc if use_bacc else bass.Bass
    nc = cls("TRN2", target_bir_lowering=False, debug=False, num_devices=N_CORES)
    xt = nc.declare_dram_parameter("xt", [C, N], F16, isOutput=False)
    wqkv = nc.declare_dram_parameter("wqkv", [C, 3 * HD], F16, isOutput=False)
    wot = nc.declare_dram_parameter("wot", [HD, O], F16, isOutput=False)
    out = nc.declare_dram_parameter("out", [N, O], FP32, isOutput=True)
    with _TC(nc, num_cores=N_CORES) as tc:
        with ExitStack() as ctx:
            pools = (
                ctx.enter_context(tc.tile_pool(name="w", bufs=2)),
                ctx.enter_context(tc.tile_pool(name="qkv", bufs=2)),
                ctx.enter_context(tc.tile_pool(name="g", bufs=16)),
                ctx.enter_context(tc.tile_pool(name="g32", bufs=8)),
                ctx.enter_context(tc.tile_pool(name="rz", bufs=8)),
                ctx.enter_context(tc.tile_pool(name="rz32", bufs=4)),
                ctx.enter_context(tc.tile_pool(name="z", bufs=2)),
                ctx.enter_context(tc.tile_pool(name="ob", bufs=2)),
                ctx.enter_context(tc.tile_pool(name="sps", bufs=3, space="PSUM")),
                ctx.enter_context(tc.tile_pool(name="awps", bufs=1, space="PSUM")),
            )
            args = (tc, pools, xt.ap(), wqkv.ap(), wot.ap(), out.ap())
            trips, rem = divmod(loop, unroll)
            if loop == 0:
                _emit_body(*args)
            else:
                if trips:
                    with tc.For_i(0, trips, 1):
                        for _ in range(unroll):
                            _emit_body(*args)
                for _ in range(rem):
                    _emit_body(*args)
    return nc


def make_in_maps(features, weight_q, weight_k, weight_v, weight_out):
    wqkv = np.ascontiguousarray(
        np.concatenate(
            [
                (weight_q * C_QK).reshape(C, HD),
                (weight_k * C_QK).reshape(C, HD),
                weight_v.reshape(C, HD),
            ],
            axis=1,
        ),
        dtype=np.float16,
    )
    wot = np.ascontiguousarray(weight_out.reshape(O, HD).T, dtype=np.float16)
    in_maps = []
    for b in range(B):
        xt = np.ascontiguousarray(features[b].T, dtype=np.float16)
        in_maps.append({"xt": xt, "wqkv": wqkv, "wot": wot})
    return in_maps


_CACHED_NC = None


def kernel(features, weight_q, weight_k, weight_v, weight_out):
    global _CACHED_NC
    if _CACHED_NC is None:
        _CACHED_NC = build_nc(loop=0)
    in_maps = make_in_maps(
        np.asarray(features, np.float32),
        np.asarray(weight_q, np.float32),
        np.asarray(weight_k, np.float32),
        np.asarray(weight_v, np.float32),
        np.asarray(weight_out, np.float32),
    )
    res = run_bass_kernel_spmd(_CACHED_NC, in_maps, list(range(N_CORES)))
    return np.stack([res.results[b]["out"] for b in range(B)], axis=0)


if __name__ == "__main__":
    rng = np.random.default_rng(0)
    feats = rng.standard_normal((B, N, C)).astype(np.float32)
    wq = rng.standard_normal((C, H, D)).astype(np.float32) * 0.05
    wk = rng.standard_normal((C, H, D)).astype(np.float32) * 0.05
    wv = rng.standard_normal((C, H, D)).astype(np.float32) * 0.05
    wo = rng.standard_normal((O, H, D)).astype(np.float32) * 0.05
    o = kernel(feats, wq, wk, wv, wo)
    print("kernel ran, out shape", o.shape, "finite:", np.isfinite(o).all())
